# revision 1
# baseline (speedup 1.0000x reference)
"""AttentionBlock kernel for Trainium2, 8-core SPMD.

Problem: x[2,64,64,512] -> GroupNorm(32) -> q,k,v = 1x1 conv -> attention
over the 4096 tokens of each batch image -> out = x + proj(o).

Sharding: 8 cores = 2 batches x 4 query-row blocks of 1024 rows. Each core
computes its batch's groupnorm stats and K/V (redundantly within a batch
group, avoiding any collectives) plus its own 1024 query rows of attention
and projection. The host rolls each core's x^T so its query block sits at
columns [0:1024]; attention is permutation-invariant over keys.

Per-core device algorithm (layouts chosen so nothing is ever transposed):
  - bn_stats over x^T [c, tok] -> per-channel mean/var; group-combine and
    group->channel broadcast via tiny selector matmuls -> s, t
    (hn = x*s + t is folded into the QKV weights, never materialized)
  - W' = diag(s) @ W (DVE row scale); bias rows b' = t2 @ W' via
    K=1-partition matmuls (t2 = beta/s - mu); Q/K biases are applied as
    per-partition ACT bias columns during PSUM evacuation (transposed via
    tiny ones-matmuls); the V bias is folded past attention:
    attn@(V + 1 bv'^T) = attn@V + bv', so it rides the projection bias
  - QT[c,1024], KT[c,4096]: W' stationary, x^T moving; V[4096,c]: x^T
    stationary, W' moving
  - single scores pass at N=1024 into bf16 PSUM; Exp on ACT (no max
    subtraction: scores ~ N(0,1), max < ~6); full P^T cached in SBUF;
    rowsum rows via ones-column matmuls during the scores pass
  - two PV passes accumulate unnormalized U^T in fp32 PSUM
  - proj: out = x + (U^T.T @ Wp + rowsum*(bv'@Wp + bp)) / rowsum, with
    1/rowsum applied as a per-partition ACT scale during evacuation
"""
import os
import sys

sys.path.insert(0, "/opt/trn_rl_repo")

import numpy as np

B, H, W_, C = 2, 64, 64, 512
HW = H * W_            # 4096 tokens per batch
GROUPS, GS = 32, 16
EPS = 1e-5
P = 128
CT = C // P            # 4 channel tiles
NKJ = HW // P          # 32 key tiles
QBLK = HW // 4         # 1024 query rows per core
SCALE = float(C) ** -0.5
N_QSUB = QBLK // 512   # 2 qi sub-blocks of 512

MM_DT_NAME = os.environ.get("KMM_DT", "bfloat16")


def build_kernel():
    import concourse.mybir as mybir
    import concourse.tile as tile
    from concourse import bacc

    f32 = mybir.dt.float32
    use_bf16 = MM_DT_NAME == "bfloat16"
    mm_store = mybir.dt.bfloat16 if use_bf16 else f32

    def mmd(ap):
        if use_bf16:
            return ap
        return ap.bitcast(mybir.dt.float32r)

    nc = bacc.Bacc("TRN2", target_bir_lowering=False)

    xT = nc.dram_tensor("xT", [C, HW], f32, kind="ExternalInput")
    xq = nc.dram_tensor("xq", [QBLK, C], f32, kind="ExternalInput")
    xnat = nc.dram_tensor("xn", [HW, C], f32, kind="ExternalInput")
    Wd = {n: nc.dram_tensor(n, [C, C], f32, kind="ExternalInput")
          for n in ("Wq", "Wk", "Wv", "Wp")}
    bd = {n: nc.dram_tensor(n, [1, C], f32, kind="ExternalInput")
          for n in ("bq", "bk", "bv", "bp")}
    gammaT = nc.dram_tensor("gammaT", [C, 1], f32, kind="ExternalInput")
    betaT = nc.dram_tensor("betaT", [C, 1], f32, kind="ExternalInput")
    gsel = nc.dram_tensor("gsel", [C, GROUPS], f32, kind="ExternalInput")
    gexp = nc.dram_tensor("gexp", [GROUPS, C], f32, kind="ExternalInput")
    out = nc.dram_tensor("out", [QBLK, C], f32, kind="ExternalOutput")

    xTr = xT.rearrange("(t p) n -> p t n", p=P)       # [128, 4, 4096]
    Exp = mybir.ActivationFunctionType.Exp
    Sqrt = mybir.ActivationFunctionType.Sqrt
    Copy = mybir.ActivationFunctionType.Copy
    Ident = mybir.ActivationFunctionType.Identity
    MUL = mybir.AluOpType.mult
    ADD = mybir.AluOpType.add
    SUB = mybir.AluOpType.subtract

    with tile.TileContext(nc) as tc:
        mm = nc.tensor.matmul

        # ---------------- persistent tensors ----------------
        persist = tc.alloc_tile_pool(name="persist", bufs=1)
        kt = persist.tile([P, CT, HW], mm_store, name="kt")       # K^T
        xn = persist.tile([P, NKJ, C], mm_store, name="xn")       # x natural, kj-major
        qt = persist.tile([P, CT, QBLK], mm_store, name="qt")     # Q^T
        ut = persist.tile([P, CT, QBLK], mm_store, name="ut")     # U^T unnorm
        wp_t = persist.tile([P, CT, C], mm_store, name="wp_t")
        ones_mm = persist.tile([P, 512], mm_store, name="ones_mm")
        ones_f32 = persist.tile([P, 1], f32, name="ones_f32")
        eps_t = persist.tile([P, 1], f32, name="eps_t")
        gma = persist.tile([P, CT], f32, name="gma")
        bta = persist.tile([P, CT], f32, name="bta")
        gsel_t = persist.tile([P, CT, GROUPS], f32, name="gsel_t")
        gexp_t = persist.tile([GROUPS, CT, P], f32, name="gexp_t")
        st_s = persist.tile([P, CT], f32, name="st_s")            # fold scale s
        tmm = persist.tile([P, CT], mm_store, name="tmm")         # t2
        bqc = persist.tile([P, CT], f32, name="bqc")              # bias columns
        bkc = persist.tile([P, CT], f32, name="bkc")
        bvc = persist.tile([P, CT], mm_store, name="bvc")         # bv' col, mm dt
        bp_row = persist.tile([1, C], f32, name="bp_row")
        bvp_sb = persist.tile([1, C], mm_store, name="bvp_sb")    # bv'@Wp + bp
        rsr = persist.tile([P, 2 * CT], f32, name="rsr")          # 1/rowsum cols
        rs_mm = persist.tile([1, QBLK], mm_store, name="rs_mm")

        warm = persist.tile([P, 1], f32, name="warm")
        nc.gpsimd.memset(ones_mm, 1.0)
        nc.vector.memset(ones_f32, 1.0)
        nc.vector.memset(eps_t, EPS)
        nc.scalar.activation(out=warm, in_=eps_t, func=Sqrt)  # pre-warm table
        nc.sync.dma_start(out=gma, in_=gammaT.rearrange("(t p) o -> p (t o)", p=P))
        nc.sync.dma_start(out=bta, in_=betaT.rearrange("(t p) o -> p (t o)", p=P))
        nc.sync.dma_start(out=gsel_t, in_=gsel.rearrange("(t p) g -> p t g", p=P))
        nc.sync.dma_start(out=gexp_t, in_=gexp.rearrange("g (t p) -> g t p", p=P))
        nc.sync.dma_start(out=bp_row, in_=bd["bp"][0:1, :])

        # x^T resident only until V is built; P^T cache reuses its space
        xrp = tc.alloc_tile_pool(name="xrp", bufs=1)
        xr = xrp.tile([P, CT, HW], mm_store, name="xr")

        # ---------------- stats ----------------
        stats = tc.alloc_tile_pool(name="stats", bufs=1)
        bst = stats.tile([P, CT, 8, 6], f32, name="bst")
        mv = stats.tile([P, CT, 2], f32, name="mv")
        rhs2 = stats.tile([P, CT, 2], f32, name="rhs2")
        gst = stats.tile([GROUPS, 4], f32, name="gst")
        t2f = stats.tile([P, CT], f32, name="t2f")
        sinv = stats.tile([P, CT], f32, name="sinv")

        dma_x = nc.gpsimd.dma_start if use_bf16 else nc.sync.dma_start
        for ci in range(CT):
            for ch in range(8):
                sl = slice(ch * 512, (ch + 1) * 512)
                dma_x(out=xr[:, ci, sl], in_=xTr[:, ci, sl])
                nc.vector.bn_stats(out=bst[:, ci, ch, :], in_=xr[:, ci, sl])
            nc.vector.bn_aggr(out=mv[:, ci, :], in_=bst[:, ci, :, :])
            nc.vector.tensor_copy(rhs2[:, ci, 0:1], mv[:, ci, 0:1])
            nc.vector.tensor_tensor(out=rhs2[:, ci, 1:2], in0=mv[:, ci, 0:1],
                                    in1=mv[:, ci, 0:1], op=MUL)
            nc.vector.tensor_tensor(out=rhs2[:, ci, 1:2], in0=rhs2[:, ci, 1:2],
                                    in1=mv[:, ci, 1:2], op=ADD)

        smalls = tc.alloc_tile_pool(name="smalls", bufs=1, space="PSUM")
        gs_ps = smalls.tile([GROUPS, 2], f32, name="gs_ps", tag="gs")
        for ci in range(CT):
            mm(gs_ps, lhsT=gsel_t[:, ci, :], rhs=rhs2[:, ci, :],
               start=(ci == 0), stop=(ci == CT - 1), skip_group_check=True)
        # gst columns: 0=rstd_g 1=mu_g 2=E[x^2]->var_g 3=scratch
        nc.vector.tensor_copy(gst[:, 1:3], gs_ps[:, 0:2])
        nc.vector.tensor_tensor(out=gst[:, 3:4], in0=gst[:, 1:2],
                                in1=gst[:, 1:2], op=MUL)
        nc.vector.tensor_tensor(out=gst[:, 2:3], in0=gst[:, 2:3],
                                in1=gst[:, 3:4], op=SUB)
        nc.scalar.activation(out=gst[:, 3:4], in_=gst[:, 2:3], func=Sqrt,
                             bias=eps_t[0:GROUPS, :], scale=1.0)
        nc.vector.reciprocal(out=gst[:, 0:1], in_=gst[:, 3:4])

        cb_all = smalls.tile([P, CT, 2], f32, name="cb_all", tag="cb")
        for ci in range(CT):
            mm(cb_all[:, ci, :], lhsT=gexp_t[:, ci, :], rhs=gst[:, 0:2],
               start=(ci == 0), stop=(ci == CT - 1), skip_group_check=True)
        nc.vector.tensor_tensor(out=st_s, in0=cb_all[:, :, 0], in1=gma, op=MUL)
        # ---------------- fold weights, build bias columns ----------------
        wpool = tc.alloc_tile_pool(name="wpool", bufs=1)
        w_t = {n: wpool.tile([P, CT, C], mm_store, name=f"{n}_t")
               for n in ("Wq", "Wk", "Wv")}
        w_t["Wp"] = wp_t
        wraw = tc.alloc_tile_pool(name="wraw", bufs=5)

        def transpose_row(row_f32, col_ps):
            """[1,512] f32 row -> [128,4] psum column via 4 tiny fp32 mms."""
            for j in range(CT):
                mm(col_ps[:, j:j + 1], lhsT=row_f32[0:1, j * P:(j + 1) * P],
                   rhs=ones_f32[0:1, 0:1],
                   start=(j == 0), stop=(j == CT - 1), skip_group_check=True)

        def fold_weight(wn):
            wt = w_t[wn]
            wdr = Wd[wn].rearrange("(t p) n -> p t n", p=P)
            for ci in range(CT):
                w_raw = wraw.tile([P, 512], f32, name="w_raw", tag="wr")
                nc.sync.dma_start(out=w_raw, in_=wdr[:, ci, :])
                if wn == "Wp":
                    nc.vector.tensor_copy(wt[:, ci, :], w_raw)
                elif wn == "Wq":
                    nc.vector.tensor_scalar(out=wt[:, ci, :], in0=w_raw,
                                            scalar1=st_s[:, ci:ci + 1],
                                            scalar2=SCALE, op0=MUL, op1=MUL)
                else:
                    nc.vector.tensor_scalar_mul(wt[:, ci, :], in0=w_raw,
                                                scalar1=st_s[:, ci:ci + 1])

        fold_weight("Wk")          # K build starts after these 4 folds
        nc.vector.reciprocal(out=sinv, in_=st_s)
        nc.vector.tensor_tensor(out=t2f, in0=bta, in1=sinv, op=MUL)
        nc.vector.tensor_tensor(out=t2f, in0=t2f, in1=cb_all[:, :, 1], op=SUB)
        nc.vector.tensor_copy(tmm, t2f)
        fold_weight("Wq")
        fold_weight("Wv")
        fold_weight("Wp")

        def bias_chain(wn, bn, dstc):
            """b' = t2 @ W' (+ b_raw, q scaled) -> per-partition column."""
            wt = w_t[wn]
            b_raw = wraw.tile([1, C], f32, name="b_raw", tag="br")
            nc.sync.dma_start(out=b_raw, in_=bd[bn][0:1, :])
            bps = smalls.tile([1, C], f32, name="bps", tag="bps")
            for ci in range(CT):
                mm(bps, lhsT=mmd(tmm[:, ci:ci + 1]), rhs=mmd(wt[:, ci, :]),
                   start=(ci == 0), stop=(ci == CT - 1), skip_group_check=True)
            if wn == "Wq":
                nc.vector.tensor_scalar_mul(b_raw, in0=b_raw, scalar1=SCALE)
            brow = wraw.tile([1, C], f32, name="brow", tag="brow")
            nc.vector.tensor_tensor(out=brow, in0=bps, in1=b_raw, op=ADD)
            bcol_ps = smalls.tile([P, CT], f32, name="bcol_ps", tag="bcol")
            transpose_row(brow, bcol_ps)
            nc.vector.tensor_copy(dstc, bcol_ps)

        # ---------------- build V, biases, QT, KT ----------------
        qkv_ps = tc.alloc_tile_pool(name="qkv_ps", bufs=4, space="PSUM")

        def lin_evac(dst, wt, bcol, co, x0):
            ps = qkv_ps.tile([P, 512], f32, name="ps", tag="qkv")
            cosl = slice(co * P, (co + 1) * P)
            for ci in range(CT):
                mm(ps, lhsT=mmd(wt[:, ci, cosl]), rhs=mmd(xr[:, ci, x0:x0 + 512]),
                   start=(ci == 0), stop=(ci == CT - 1), skip_group_check=True)
            if bcol is None:
                nc.scalar.activation(out=dst, in_=ps, func=Copy)
            else:
                nc.scalar.activation(out=dst, in_=ps, func=Ident,
                                     bias=bcol[:, co:co + 1], scale=1.0)

        nc.scalar.activation(out=warm, in_=eps_t, func=Exp)  # pre-warm table
        for co in range(CT):
            for kf in range(8):
                lin_evac(kt[:, co, kf * 512:(kf + 1) * 512], w_t["Wk"],
                         None, co, kf * 512)
            if co == 0:
                bias_chain("Wk", "bk", bkc)
            elif co == 1:
                bias_chain("Wq", "bq", bqc)
            elif co == 2:
                bias_chain("Wv", "bv", bvc)
                # bvp = bv' @ Wp + bp (V bias folded past attention)
                bvp_ps = smalls.tile([1, C], f32, name="bvp_ps", tag="bps")
                for ci in range(CT):
                    mm(bvp_ps, lhsT=mmd(bvc[:, ci:ci + 1]),
                       rhs=mmd(wp_t[:, ci, :]),
                       start=(ci == 0), stop=(ci == CT - 1),
                       skip_group_check=True)
                nc.vector.tensor_tensor(out=bvp_sb, in0=bvp_ps, in1=bp_row,
                                        op=ADD)
        for co in range(CT):
            nc.vector.tensor_scalar(out=kt[:, co, :], in0=kt[:, co, :],
                                    scalar1=bkc[:, co:co + 1], scalar2=None,
                                    op0=ADD)
        for co in range(CT):
            for qf in range(N_QSUB):
                lin_evac(qt[:, co, qf * 512:(qf + 1) * 512], w_t["Wq"],
                         None, co, qf * 512)
            nc.vector.tensor_scalar(out=qt[:, co, :], in0=qt[:, co, :],
                                    scalar1=bqc[:, co:co + 1], scalar2=None,
                                    op0=ADD)
        xnr = xnat.rearrange("(t p) c -> p t c", p=P)
        for t in range(8):
            dma_x(out=xn[:, 4 * t:4 * t + 4, :], in_=xnr[:, 4 * t:4 * t + 4, :])

        qkv_ps.release()
        smalls.release()
        wraw.release()
        wpool.release()
        stats.release()
        xrp.release()

        # ---------------- attention + projection ----------------
        # Z = P @ x accumulated per block; U^T = Wv'^T Z afterwards
        o_ps_pool = tc.alloc_tile_pool(name="o_ps", bufs=1, space="PSUM")
        s_ps_pool = tc.alloc_tile_pool(name="s_ps", bufs=3, space="PSUM")
        rs_ps_pool = tc.alloc_tile_pool(name="rs_ps", bufs=1, space="PSUM")
        pt_pool = tc.alloc_tile_pool(name="pt", bufs=8)
        rssb_pool = tc.alloc_tile_pool(name="rssb", bufs=2)
        zsb_pool = tc.alloc_tile_pool(name="zsb", bufs=2)
        res_pool = tc.alloc_tile_pool(name="res", bufs=3)
        out_pool = tc.alloc_tile_pool(name="outp", bufs=3)

        for qb in range(N_QSUB):
            qsl = slice(qb * 512, (qb + 1) * 512)
            z_tiles = [o_ps_pool.tile([P, 512], f32, name=f"o{ci}", tag=f"o{ci}")
                       for ci in range(CT)]
            rs_ps = rs_ps_pool.tile([1, 512], f32, name="rs_ps", tag="rs")

            def scores(kj):
                s_ps = s_ps_pool.tile([P, 512], f32, name="s_ps", tag="s")
                for ci in range(CT):
                    mm(s_ps, lhsT=mmd(kt[:, ci, kj * P:(kj + 1) * P]),
                       rhs=mmd(qt[:, ci, qsl]),
                       start=(ci == 0), stop=(ci == CT - 1), skip_group_check=True)
                pt = pt_pool.tile([P, 512], mm_store, name="pt", tag="pt")
                nc.scalar.activation(out=pt, in_=s_ps, func=Exp)
                return pt

            def accum(kj, pt):
                mm(rs_ps, lhsT=mmd(ones_mm[:, 0:1]), rhs=mmd(pt),
                   start=(kj == 0), stop=(kj == NKJ - 1), skip_group_check=True)
                for ci in range(CT):
                    mm(z_tiles[ci], lhsT=mmd(xn[:, kj, ci * P:(ci + 1) * P]),
                       rhs=mmd(pt),
                       start=(kj == 0), stop=(kj == NKJ - 1),
                       skip_group_check=True)

            pt_prev = scores(0)
            for kj in range(1, NKJ):
                pt_cur = scores(kj)
                accum(kj - 1, pt_prev)
                pt_prev = pt_cur
            accum(NKJ - 1, pt_prev)

            rs_sb = rssb_pool.tile([1, 512], f32, name="rs_sb", tag="rssb")
            nc.vector.tensor_copy(rs_sb, rs_ps)
            nc.vector.tensor_copy(rs_mm[0:1, qb * 512:(qb + 1) * 512], rs_sb)
            z_sb = zsb_pool.tile([P, CT, 512], mm_store, name="z_sb", tag="z")
            for ci in range(CT):
                nc.vector.tensor_copy(z_sb[:, ci, :], z_tiles[ci])
            rsT_ps = s_ps_pool.tile([P, 512], f32, name="rsT_ps", tag="s")
            for j in range(CT):
                mm(rsT_ps[:, j:j + 1], lhsT=rs_sb[0:1, j * P:(j + 1) * P],
                   rhs=ones_f32[0:1, 0:1],
                   start=(j == 0), stop=(j == CT - 1), skip_group_check=True)
            nc.vector.reciprocal(out=rsr[:, qb * CT:(qb + 1) * CT],
                                 in_=rsT_ps[:, 0:CT])

            # U^T = Wv'^T @ Z, reusing the Z accumulator banks
            for co in range(CT):
                u_ps = o_ps_pool.tile([P, 512], f32, name="u_ps", tag=f"o{co}")
                for ci in range(CT):
                    mm(u_ps, lhsT=mmd(w_t["Wv"][:, ci, co * P:(co + 1) * P]),
                       rhs=mmd(z_sb[:, ci, :]),
                       start=(ci == 0), stop=(ci == CT - 1),
                       skip_group_check=True)
                nc.vector.tensor_copy(ut[:, co, qsl], u_ps)

            # projection for this block rides the same banks
            for jj in range(CT):
                j = qb * CT + jj
                qi0 = j * P
                po = o_ps_pool.tile([P, 512], f32, name="po", tag=f"o{jj}")
                for ci in range(CT):
                    mm(po, lhsT=mmd(ut[:, ci, qi0:qi0 + P]),
                       rhs=mmd(wp_t[:, ci, :]),
                       start=(ci == 0), stop=False, skip_group_check=True)
                mm(po, lhsT=rs_mm[0:1, qi0:qi0 + P], rhs=mmd(bvp_sb[0:1, :]),
                   start=False, stop=True, skip_group_check=True)
                ot = out_pool.tile([P, 512], f32, name="ot", tag="ot")
                nc.vector.tensor_scalar_mul(ot, in0=po, scalar1=rsr[:, j:j + 1])
                xres = res_pool.tile([P, 512], f32, name="xres", tag="xres")
                nc.sync.dma_start(out=xres, in_=xq[qi0:qi0 + P, :])
                nc.vector.tensor_tensor(out=ot, in0=ot, in1=xres, op=ADD)
                nc.sync.dma_start(out=out[qi0:qi0 + P, :], in_=ot)

        out_pool.release()
        res_pool.release()
        zsb_pool.release()
        rssb_pool.release()
        pt_pool.release()
        rs_ps_pool.release()
        s_ps_pool.release()
        o_ps_pool.release()
        persist.release()

    nc.compile()
    return nc


_GSEL = np.kron(np.eye(GROUPS, dtype=np.float32),
                np.full((GS, 1), 1.0 / GS, np.float32))          # [512, 32]
_GEXP = np.kron(np.eye(GROUPS, dtype=np.float32),
                np.ones((1, GS), np.float32))                    # [32, 512]


def make_in_maps(x, gamma, beta, Wq, bq, Wk, bk, Wv, bv, Wp, bp):
    """Shard FULL inputs into 8 per-core input dicts."""
    f = np.float32
    x = np.asarray(x, f)
    common = {
        "Wq": np.asarray(Wq, f), "Wk": np.asarray(Wk, f),
        "Wv": np.asarray(Wv, f), "Wp": np.asarray(Wp, f),
        "bq": np.asarray(bq, f).reshape(1, C), "bk": np.asarray(bk, f).reshape(1, C),
        "bv": np.asarray(bv, f).reshape(1, C), "bp": np.asarray(bp, f).reshape(1, C),
        "gammaT": np.asarray(gamma, f).reshape(C, 1),
        "betaT": np.asarray(beta, f).reshape(C, 1),
        "gsel": _GSEL, "gexp": _GEXP,
    }
    in_maps = []
    for b in range(B):
        xb = x[b].reshape(HW, C)
        xTb = np.ascontiguousarray(xb.T)                         # [512, 4096]
        for qb in range(4):
            xTroll = np.ascontiguousarray(np.roll(xTb, -qb * QBLK, axis=1))
            m = dict(common)
            m["xT"] = xTroll
            m["xn"] = np.ascontiguousarray(np.roll(xb, -qb * QBLK, axis=0))
            m["xq"] = np.ascontiguousarray(xb[qb * QBLK:(qb + 1) * QBLK])
            in_maps.append(m)
    return in_maps


def assemble_out(results):
    o = np.empty((B, HW, C), np.float32)
    for b in range(B):
        for qb in range(4):
            o[b, qb * QBLK:(qb + 1) * QBLK] = results[b * 4 + qb]["out"]
    return o.reshape(B, H, W_, C)


_NC_CACHE = {}


def run(inputs, trace=False, trace_cores=None):
    from concourse.bass_utils import run_bass_kernel_spmd
    key = MM_DT_NAME
    if key not in _NC_CACHE:
        _NC_CACHE[key] = build_kernel()
    nc = _NC_CACHE[key]
    in_maps = make_in_maps(**inputs)
    res = run_bass_kernel_spmd(nc, in_maps, core_ids=list(range(8)),
                               trace=trace, trace_cores=trace_cores)
    return assemble_out(res.results), res


def kernel(**inputs) -> np.ndarray:
    out, _ = run(inputs, trace=False)
    return out



# revision 6
# speedup vs baseline: 1.6240x; 1.6240x over previous
"""AttentionBlock kernel for Trainium2, 8-core SPMD, fp8 DoubleRow edition.

Problem: x[2,64,64,512] -> GroupNorm(32) -> q,k,v = 1x1 conv -> attention
over the 4096 tokens of each batch image -> out = x + proj(o).

Sharding: 8 cores = 2 batches x 4 query-row blocks of 1024 rows. The host
rolls each core's x so its query block sits at rows [0:1024]; attention is
permutation-invariant over keys. Host pre-casts x and weights to fp8/bf16.

Math restructure vs a direct port (all biases/affine exact):
  - scores^T[j,i] = x_j . R_i with R = diag(s) Wk q^T and q^T built from
    Wq''= diag(s)*Wq*sc (device fold) against raw fp8 x^T. No K tensor is
    ever built (saves a redundant 4096x512x512 matmul per core) and the
    k-bias bk drops entirely (constant per query row -> cancels in softmax,
    as does t.Wk.q).
  - exp uses a global -2 shift to keep e4m3 range; rowsum normalization
    cancels it exactly.
  - Z = P @ x_raw (fp8 DoubleRow); V never materialized:
    attnV_unnorm = Wv^T(s*Z) + rowsum*(t.Wv + bv), the rowsum term rides a
    rank-1 bf16 matmul into the projection PSUM.
  - All heavy matmuls are fp8e4 DoubleRow (2 k-tiles per instruction).
    Scale plan: FW=16 on host weights, FQ=256 Wq fold, qt/R stored x16,
    z stored as s*Z/4, ut = Uu/2, proj psum = 8*Uu@Wp, evac scale 1/(8*rs).
"""
import os
import sys

sys.path.insert(0, "/opt/trn_rl_repo")

import numpy as np
import ml_dtypes

B, H, W_, C = 2, 64, 64, 512
HW = H * W_            # 4096 tokens per batch
GROUPS, GS = 32, 16
EPS = 1e-5
P = 128
CT = C // P            # 4 channel tiles
NKJ = HW // P          # 32 key tiles
NPAIR = NKJ // 2       # 16 DoubleRow key-tile pairs
QBLK = HW // 4         # 1024 query rows per core
SCALE = float(C) ** -0.5
N_QSUB = QBLK // 512   # 2 qi sub-blocks of 512

FW = 16.0              # host weight pre-scale (fp8 range)
FQ = 256.0             # Wq'' fold scale
FQT = 16.0             # qt storage scale
FR = 16.0              # R storage scale
FZ = 0.25              # z storage scale (s*Z/4)
FU = 0.125             # ut storage scale (Uu/8)
FP_PO = FU * FW        # proj psum carries FP_PO * Uu@Wp = 8x
EXP_SHIFT = -2.0

MM_DT_NAME = "fp8dr"

N_WARM = 64            # dummy PE matmuls paced by x chunks (HAM warmth)


def build_kernel():
    import concourse.mybir as mybir
    import concourse.tile as tile
    from concourse import bacc

    f32 = mybir.dt.float32
    bf16 = mybir.dt.bfloat16
    f8 = mybir.dt.float8e4
    DR = mybir.MatmulPerfMode.DoubleRow

    nc = bacc.Bacc("TRN2", target_bir_lowering=False)

    xT8d = nc.dram_tensor("xT8", [C, HW], f8, kind="ExternalInput")
    xn8d = nc.dram_tensor("xn8", [HW, C], f8, kind="ExternalInput")
    xqd = nc.dram_tensor("xq", [QBLK, C], f32, kind="ExternalInput")
    wkT8d = nc.dram_tensor("WkT8", [C, C], f8, kind="ExternalInput")
    wv8d = nc.dram_tensor("Wv8", [C, C], f8, kind="ExternalInput")
    wp8d = nc.dram_tensor("Wp8", [C, C], f8, kind="ExternalInput")
    wq16d = nc.dram_tensor("Wq16", [C, C], bf16, kind="ExternalInput")
    wv16d = nc.dram_tensor("Wv16", [C, C], bf16, kind="ExternalInput")
    wp16d = nc.dram_tensor("Wp16", [C, C], bf16, kind="ExternalInput")
    bqd = nc.dram_tensor("bq", [1, C], f32, kind="ExternalInput")
    bvd = nc.dram_tensor("bv", [1, C], f32, kind="ExternalInput")
    bpd = nc.dram_tensor("bp", [1, C], f32, kind="ExternalInput")
    gammaT = nc.dram_tensor("gammaT", [C, 1], f32, kind="ExternalInput")
    betaT = nc.dram_tensor("betaT", [C, 1], f32, kind="ExternalInput")
    gseld = nc.dram_tensor("gsel", [C, GROUPS], f32, kind="ExternalInput")
    gexpd = nc.dram_tensor("gexp", [GROUPS, C], f32, kind="ExternalInput")
    ones8d = nc.dram_tensor("ones8", [P, P], f8, kind="ExternalInput")
    outd = nc.dram_tensor("out", [QBLK, C], f32, kind="ExternalOutput")

    xT8r = xT8d.rearrange("(t p) n -> p t n", p=P)     # [128, 4, 4096]
    xn8r = xn8d.rearrange("(t p) c -> p t c", p=P)     # [128, 32, 512]
    wkT8r = wkT8d.rearrange("(t p) n -> p t n", p=P)
    wv8r = wv8d.rearrange("(t p) n -> p t n", p=P)
    wp8r = wp8d.rearrange("(t p) n -> p t n", p=P)
    wq16r = wq16d.rearrange("(t p) n -> p t n", p=P)
    wv16r = wv16d.rearrange("(t p) n -> p t n", p=P)
    wp16r = wp16d.rearrange("(t p) n -> p t n", p=P)

    Exp = mybir.ActivationFunctionType.Exp
    Sqrt = mybir.ActivationFunctionType.Sqrt
    Copy = mybir.ActivationFunctionType.Copy
    Ident = mybir.ActivationFunctionType.Identity
    MUL = mybir.AluOpType.mult
    ADD = mybir.AluOpType.add
    SUB = mybir.AluOpType.subtract

    with tile.TileContext(nc) as tc:
        mm = nc.tensor.matmul

        # ---------------- persistent tensors ----------------
        persist = tc.alloc_tile_pool(name="persist", bufs=1)
        xt8 = persist.tile([P, CT, HW], f8, name="xt8")        # x^T fp8
        xn8 = persist.tile([P, NKJ, C], f8, name="xn8")        # x natural fp8
        qt8 = persist.tile([P, CT, QBLK], f8, name="qt8")      # FQT * q^T
        r8 = persist.tile([P, CT, QBLK], f8, name="r8")        # FR * R
        ut8 = persist.tile([P, CT, QBLK], f8, name="ut8")      # FU * Uu^T
        z8 = persist.tile([P, CT, 512], f8, name="z8")         # FZ * s*Z
        wk8 = persist.tile([P, CT, C], f8, name="wk8")         # host FW*Wk^T
        wv8 = persist.tile([P, CT, C], f8, name="wv8")
        wp8 = persist.tile([P, CT, C], f8, name="wp8")
        wq8 = persist.tile([P, CT, C], f8, name="wq8")         # device fold
        ones8 = persist.tile([P, 2, 16], f8, name="ones8")     # DR rowsum lhsT
        onesq8 = persist.tile([P, P], f8, name="onesq8")       # warm lhsT
        c1 = persist.tile([P, 1], f32, name="c1")
        c8 = persist.tile([P, 1], f32, name="c8")
        eps_t = persist.tile([P, 1], f32, name="eps_t")
        gma = persist.tile([P, CT], f32, name="gma")
        bta = persist.tile([P, CT], f32, name="bta")
        gsel_t = persist.tile([P, CT, GROUPS], f32, name="gsel_t")
        gexp_t = persist.tile([GROUPS, CT, P], f32, name="gexp_t")
        st_s = persist.tile([P, CT], f32, name="st_s")         # s = gamma*rstd
        tmm = persist.tile([P, CT], bf16, name="tmm")          # t (bf16)
        foldq = persist.tile([P, CT], f32, name="foldq")       # s*SCALE*FQ
        rcol = persist.tile([P, CT], f32, name="rcol")         # s/FQ... s*FR/(FW*FQT)
        zcol = persist.tile([P, CT], f32, name="zcol")         # s*FZ
        v0col = persist.tile([P, CT], f32, name="v0col")       # FQT*v0
        brow8 = persist.tile([1, C], bf16, name="brow8")       # FP_PO*(bvt@Wp+bp)
        rs_mm = persist.tile([1, QBLK], bf16, name="rs_mm")    # rowsums bf16
        rsr = persist.tile([P, N_QSUB * CT], f32, name="rsr")  # 1/(8*rs) cols
        xres = persist.tile([P, 2 * CT, C], f32, name="xres")  # residual x rows
        neg2 = persist.tile([P, 1], f32, name="neg2")
        warm_sb = persist.tile([P, 1], f32, name="warm_sb")

        nc.vector.memset(c1, 1.0)
        nc.vector.memset(c8, FP_PO)
        nc.vector.memset(eps_t, EPS)
        nc.vector.memset(neg2, EXP_SHIFT)
        # prewarm ACT tables (order irrelevant; loaded once per func)
        nc.scalar.activation(out=warm_sb, in_=eps_t, func=Exp)
        nc.scalar.activation(out=warm_sb, in_=eps_t, func=Sqrt)

        nc.gpsimd.dma_start(out=ones8, in_=ones8d[0:P, 0:32])
        nc.gpsimd.dma_start(out=onesq8, in_=ones8d[:, :])
        nc.gpsimd.dma_start(out=gma, in_=gammaT.rearrange("(t p) o -> p (t o)", p=P))
        nc.gpsimd.dma_start(out=bta, in_=betaT.rearrange("(t p) o -> p (t o)", p=P))
        nc.gpsimd.dma_start(out=gsel_t, in_=gseld.rearrange("(t p) g -> p t g", p=P))
        nc.gpsimd.dma_start(out=gexp_t, in_=gexpd.rearrange("g (t p) -> g t p", p=P))

        # weight / residual DMAs (gpsimd queue, overlap the xT8 stream)
        wq16 = persist.tile([P, CT, C], bf16, name="wq16")
        wv16 = persist.tile([P, CT, C], bf16, name="wv16")
        wp16 = persist.tile([P, CT, C], bf16, name="wp16")
        bq_row = persist.tile([1, C], f32, name="bq_row")
        bv_row = persist.tile([1, C], f32, name="bv_row")
        bp_row = persist.tile([1, C], f32, name="bp_row")
        nc.gpsimd.dma_start(out=wk8, in_=wkT8r[:, :, :])
        nc.gpsimd.dma_start(out=wq16, in_=wq16r[:, :, :])
        nc.gpsimd.dma_start(out=wv8, in_=wv8r[:, :, :])
        nc.gpsimd.dma_start(out=wp8, in_=wp8r[:, :, :])
        nc.gpsimd.dma_start(out=wv16, in_=wv16r[:, :, :])
        nc.gpsimd.dma_start(out=wp16, in_=wp16r[:, :, :])
        nc.gpsimd.dma_start(out=bq_row, in_=bqd[0:1, :])
        nc.gpsimd.dma_start(out=bv_row, in_=bvd[0:1, :])
        nc.gpsimd.dma_start(out=bp_row, in_=bpd[0:1, :])
        for j in range(2 * CT):
            nc.scalar.dma_start(out=xres[:, j, :], in_=xqd[j * P:(j + 1) * P, :])

        # ---------------- stats (+ PE warm dummies paced by chunks) -------
        stats = tc.alloc_tile_pool(name="stats", bufs=1)
        bst = stats.tile([P, CT, 8, 6], f32, name="bst")
        mv = stats.tile([P, CT, 2], f32, name="mv")
        rhs2 = stats.tile([P, CT, 2], f32, name="rhs2")
        gst = stats.tile([GROUPS, 4], f32, name="gst")

        warm_pool = tc.alloc_tile_pool(name="warmp", bufs=1, space="PSUM")
        warm_ps = warm_pool.tile([P, 512], f32, name="warm_ps", tag="warm")
        n_chunks = CT * 8
        for ci in range(CT):
            for ch in range(8):
                sl = slice(ch * 512, (ch + 1) * 512)
                nc.sync.dma_start(out=xt8[:, ci, sl], in_=xT8r[:, ci, sl])
                nc.vector.bn_stats(out=bst[:, ci, ch, :], in_=xt8[:, ci, sl])
                k = ci * 8 + ch
                for r in range(N_WARM // n_chunks):
                    mm(warm_ps, lhsT=onesq8, rhs=xt8[:, ci, sl],
                       start=(k == 0 and r == 0),
                       stop=(k == n_chunks - 1 and r == N_WARM // n_chunks - 1),
                       skip_group_check=True)
            nc.vector.bn_aggr(out=mv[:, ci, :], in_=bst[:, ci, :, :])
            nc.vector.tensor_copy(rhs2[:, ci, 0:1], mv[:, ci, 0:1])
            nc.vector.tensor_tensor(out=rhs2[:, ci, 1:2], in0=mv[:, ci, 0:1],
                                    in1=mv[:, ci, 0:1], op=MUL)
            nc.vector.tensor_tensor(out=rhs2[:, ci, 1:2], in0=rhs2[:, ci, 1:2],
                                    in1=mv[:, ci, 1:2], op=ADD)
        nc.scalar.activation(out=warm_sb, in_=warm_ps[:, 0:1], func=Copy)

        smalls = tc.alloc_tile_pool(name="smalls", bufs=1, space="PSUM")
        gs_ps = smalls.tile([GROUPS, 2], f32, name="gs_ps", tag="gs")
        for ci in range(CT):
            mm(gs_ps, lhsT=gsel_t[:, ci, :], rhs=rhs2[:, ci, :],
               start=(ci == 0), stop=(ci == CT - 1), skip_group_check=True)
        # gst columns: 0=rstd_g 1=mu_g 2=var_g 3=scratch
        nc.vector.tensor_copy(gst[:, 1:3], gs_ps[:, 0:2])
        nc.vector.tensor_tensor(out=gst[:, 3:4], in0=gst[:, 1:2],
                                in1=gst[:, 1:2], op=MUL)
        nc.vector.tensor_tensor(out=gst[:, 2:3], in0=gst[:, 2:3],
                                in1=gst[:, 3:4], op=SUB)
        nc.scalar.activation(out=gst[:, 3:4], in_=gst[:, 2:3], func=Sqrt,
                             bias=eps_t[0:GROUPS, :], scale=1.0)
        nc.vector.reciprocal(out=gst[:, 0:1], in_=gst[:, 3:4])

        cb_all = smalls.tile([P, CT, 2], f32, name="cb_all", tag="cb")
        for ci in range(CT):
            mm(cb_all[:, ci, :], lhsT=gexp_t[:, ci, :], rhs=gst[:, 0:2],
               start=(ci == 0), stop=(ci == CT - 1), skip_group_check=True)
        nc.vector.tensor_tensor(out=st_s, in0=cb_all[:, :, 0], in1=gma, op=MUL)
        # t = beta - mu_g * s   (bf16 copy for the bias-chain matmuls)
        tf32 = stats.tile([P, CT], f32, name="tf32")
        nc.vector.tensor_tensor(out=tf32, in0=cb_all[:, :, 1], in1=st_s, op=MUL)
        nc.vector.tensor_tensor(out=tf32, in0=bta, in1=tf32, op=SUB)
        nc.vector.tensor_copy(tmm, tf32)
        # evac scale columns
        nc.vector.tensor_scalar_mul(foldq, in0=st_s, scalar1=SCALE * FQ)
        nc.vector.tensor_scalar_mul(rcol, in0=st_s, scalar1=FR / (FW * FQT))
        nc.vector.tensor_scalar_mul(zcol, in0=st_s, scalar1=FZ)

        # Wq'' = s*SCALE*FQ * Wq  (bf16 -> fp8 on ACT)
        for ci in range(CT):
            nc.scalar.activation(out=wq8[:, ci, :], in_=wq16[:, ci, :],
                                 func=Copy, scale=foldq[:, ci:ci + 1])

        def transpose_row(row_f32, col_ps, rhs_const):
            """[1,512] f32 row -> [128,CT] psum column via tiny fp32 mms."""
            for j in range(CT):
                mm(col_ps[:, j:j + 1], lhsT=row_f32[0:1, j * P:(j + 1) * P],
                   rhs=rhs_const[0:1, 0:1],
                   start=(j == 0), stop=(j == CT - 1), skip_group_check=True)

        # v0 = SCALE*(Wq^T t + bq); store col = FQT*v0
        rowp = smalls.tile([1, C], f32, name="rowp", tag="rowp")
        row_q = stats.tile([1, C], f32, name="row_q")
        row_v = stats.tile([1, C], f32, name="row_v")
        for ci in range(CT):
            mm(rowp, lhsT=tmm[:, ci:ci + 1], rhs=wq16[:, ci, :],
               start=(ci == 0), stop=(ci == CT - 1), skip_group_check=True)
        nc.vector.tensor_tensor(out=row_q, in0=rowp, in1=bq_row,
                                op=ADD)
        nc.vector.tensor_scalar_mul(row_q, in0=row_q, scalar1=SCALE * FQT)
        colp = smalls.tile([P, CT], f32, name="colp", tag="colp")
        transpose_row(row_q, colp, c1)
        nc.vector.tensor_copy(v0col, colp)

        # bvt = t@Wv + bv ; brow8 = FP_PO*(bvt@Wp + bp)
        bvt_ps = smalls.tile([1, C], f32, name="bvt_ps", tag="rowp")
        for ci in range(CT):
            mm(bvt_ps, lhsT=tmm[:, ci:ci + 1], rhs=wv16[:, ci, :],
               start=(ci == 0), stop=(ci == CT - 1), skip_group_check=True)
        nc.vector.tensor_tensor(out=row_v, in0=bvt_ps, in1=bv_row,
                                op=ADD)
        bvt_colps = smalls.tile([P, CT], f32, name="bvt_colps", tag="colp")
        transpose_row(row_v, bvt_colps, c1)
        bvt_col = stats.tile([P, CT], bf16, name="bvt_col")
        nc.vector.tensor_copy(bvt_col, bvt_colps)
        brow_ps = smalls.tile([1, C], f32, name="brow_ps", tag="rowp")
        for ci in range(CT):
            mm(brow_ps, lhsT=bvt_col[:, ci:ci + 1], rhs=wp16[:, ci, :],
               start=(ci == 0), stop=(ci == CT - 1), skip_group_check=True)
        browf = stats.tile([1, C], f32, name="browf")
        nc.vector.tensor_tensor(out=browf, in0=brow_ps, in1=bp_row,
                                op=ADD)
        nc.vector.tensor_scalar_mul(browf, in0=browf, scalar1=FP_PO)
        nc.vector.tensor_copy(brow8, browf)

        # xn8 load (gpsimd queue; needed from first Z accumulation on)
        for t in range(8):
            nc.gpsimd.dma_start(out=xn8[:, 4 * t:4 * t + 4, :],
                                in_=xn8r[:, 4 * t:4 * t + 4, :])

        # ---------------- qt and R builds (fp8 DR) ----------------
        bld = tc.alloc_tile_pool(name="bld", bufs=3, space="PSUM")
        # qt^T[e, i] = sum_c Wq''[c, e] x^T[c, i]; evac: *1/FQ*FQT + FQT*v0
        for et in range(CT):
            for qf in range(N_QSUB):
                ps = bld.tile([P, 512], f32, name="qtps", tag="bld")
                qsl = slice(qf * 512, (qf + 1) * 512)
                esl = slice(et * P, (et + 1) * P)
                for cp in range(2):
                    mm(ps, lhsT=wq8[:, 2 * cp:2 * cp + 2, esl],
                       rhs=xt8[:, 2 * cp:2 * cp + 2, qsl],
                       start=(cp == 0), stop=(cp == 1),
                       perf_mode=DR, skip_group_check=True)
                nc.scalar.activation(out=qt8[:, et, qsl], in_=ps, func=Ident,
                                     bias=v0col[:, et:et + 1], scale=FQT / FQ)
        # R[c, i] = s_c/(FW*FQT)*FR * sum_e WkT[e, c] qt[e, i]
        for ct_ in range(CT):
            for qf in range(N_QSUB):
                ps = bld.tile([P, 512], f32, name="rps", tag="bld")
                qsl = slice(qf * 512, (qf + 1) * 512)
                csl = slice(ct_ * P, (ct_ + 1) * P)
                for ep in range(2):
                    mm(ps, lhsT=wk8[:, 2 * ep:2 * ep + 2, csl],
                       rhs=qt8[:, 2 * ep:2 * ep + 2, qsl],
                       start=(ep == 0), stop=(ep == 1),
                       perf_mode=DR, skip_group_check=True)
                nc.scalar.activation(out=r8[:, ct_, qsl], in_=ps, func=Copy,
                                     scale=rcol[:, ct_:ct_ + 1])

        bld.release()
        smalls.release()
        warm_pool.release()

        # ---------------- attention ----------------
        o_ps_pool = tc.alloc_tile_pool(name="o_ps", bufs=1, space="PSUM")
        s_ps_pool = tc.alloc_tile_pool(name="s_ps", bufs=3, space="PSUM")
        rs_ps_pool = tc.alloc_tile_pool(name="rs_ps", bufs=1, space="PSUM")
        pt_pool = tc.alloc_tile_pool(name="pt", bufs=4)
        rssb_pool = tc.alloc_tile_pool(name="rssb", bufs=2)
        out_pool = tc.alloc_tile_pool(name="outp", bufs=3)

        for qb in range(N_QSUB):
            qsl = slice(qb * 512, (qb + 1) * 512)
            z_tiles = [o_ps_pool.tile([P, 512], f32, name=f"o{ci}", tag=f"o{ci}")
                       for ci in range(CT)]
            rs_ps = rs_ps_pool.tile([1, 512], f32, name="rs_ps", tag="rs")

            def scores_pair(pr):
                pt = pt_pool.tile([P, 2, 512], f8, name="pt", tag="pt")
                for half in range(2):
                    kj = 2 * pr + half
                    ksl = slice(kj * P, (kj + 1) * P)
                    s_ps = s_ps_pool.tile([P, 512], f32, name="s_ps", tag="s")
                    for cp in range(2):
                        mm(s_ps, lhsT=xt8[:, 2 * cp:2 * cp + 2, ksl],
                           rhs=r8[:, 2 * cp:2 * cp + 2, qsl],
                           start=(cp == 0), stop=(cp == 1),
                           perf_mode=DR, skip_group_check=True)
                    nc.scalar.activation(out=pt[:, half, :], in_=s_ps,
                                         func=Exp, scale=1.0 / FR, bias=neg2)
                return pt

            def accum(pr, pt):
                mm(rs_ps, lhsT=ones8[:, :, 0:1], rhs=pt[:, :, :],
                   start=(pr == 0), stop=(pr == NPAIR - 1),
                   perf_mode=DR, skip_group_check=True)
                for ci in range(CT):
                    mm(z_tiles[ci],
                       lhsT=xn8[:, 2 * pr:2 * pr + 2, ci * P:(ci + 1) * P],
                       rhs=pt[:, :, :],
                       start=(pr == 0), stop=(pr == NPAIR - 1),
                       perf_mode=DR, skip_group_check=True)

            pt_prev = scores_pair(0)
            for pr in range(1, NPAIR):
                pt_cur = scores_pair(pr)
                accum(pr - 1, pt_prev)
                pt_prev = pt_cur
            accum(NPAIR - 1, pt_prev)

            # rowsum -> bf16 row + 1/(FP_PO*rs) column
            rs_sb = rssb_pool.tile([1, 512], f32, name="rs_sb", tag="rssb")
            nc.vector.tensor_copy(rs_sb, rs_ps)
            nc.vector.tensor_copy(rs_mm[0:1, qsl], rs_sb)
            rsT_ps = s_ps_pool.tile([P, 512], f32, name="rsT_ps", tag="s")
            transpose_row(rs_sb, rsT_ps[:, 0:CT], c8)
            nc.vector.reciprocal(out=rsr[:, qb * CT:(qb + 1) * CT],
                                 in_=rsT_ps[:, 0:CT])

            # z8 = s*Z/4 (fp8)
            for ci in range(CT):
                nc.scalar.activation(out=z8[:, ci, :], in_=z_tiles[ci],
                                     func=Copy, scale=zcol[:, ci:ci + 1])

            # Uu^T = Wv^T (s*Z): psum = FW*FZ*Uu = 4*Uu; store FU*Uu
            for co in range(CT):
                u_ps = o_ps_pool.tile([P, 512], f32, name="u_ps", tag=f"o{co}")
                for cp in range(2):
                    mm(u_ps, lhsT=wv8[:, 2 * cp:2 * cp + 2, co * P:(co + 1) * P],
                       rhs=z8[:, 2 * cp:2 * cp + 2, :],
                       start=(cp == 0), stop=(cp == 1),
                       perf_mode=DR, skip_group_check=True)
                nc.scalar.activation(out=ut8[:, co, qsl], in_=u_ps,
                                     func=Copy, scale=FU / (FW * FZ))

            # projection: po = FU*FW*(Uu@Wp) + rank-1 rowsum bias
            for jj in range(CT):
                j = qb * CT + jj
                qi0 = j * P
                po = o_ps_pool.tile([P, 512], f32, name="po", tag=f"o{jj}")
                for cp in range(2):
                    mm(po, lhsT=ut8[:, 2 * cp:2 * cp + 2, qi0:qi0 + P],
                       rhs=wp8[:, 2 * cp:2 * cp + 2, :],
                       start=(cp == 0), stop=False,
                       perf_mode=DR, skip_group_check=True)
                mm(po, lhsT=rs_mm[0:1, qi0:qi0 + P], rhs=brow8[0:1, :],
                   start=False, stop=True, skip_group_check=True)
                ot = out_pool.tile([P, 512], f32, name="ot", tag="ot")
                nc.vector.tensor_scalar_mul(ot, in0=po, scalar1=rsr[:, j:j + 1])
                nc.vector.tensor_tensor(out=ot, in0=ot, in1=xres[:, j, :],
                                        op=ADD)
                nc.sync.dma_start(out=outd[qi0:qi0 + P, :], in_=ot)

        out_pool.release()
        rssb_pool.release()
        pt_pool.release()
        rs_ps_pool.release()
        s_ps_pool.release()
        o_ps_pool.release()
        stats.release()
        persist.release()

    nc.compile()
    return nc


_GSEL = np.kron(np.eye(GROUPS, dtype=np.float32),
                np.full((GS, 1), 1.0 / GS, np.float32))          # [512, 32]
_GEXP = np.kron(np.eye(GROUPS, dtype=np.float32),
                np.ones((1, GS), np.float32))                    # [32, 512]


def make_in_maps(x, gamma, beta, Wq, bq, Wk, bk, Wv, bv, Wp, bp):
    """Shard FULL inputs into 8 per-core input dicts (host casts fp8/bf16)."""
    f = np.float32
    f8 = ml_dtypes.float8_e4m3
    b16 = ml_dtypes.bfloat16
    x = np.asarray(x, f)
    Wq, Wk, Wv, Wp = (np.asarray(w, f) for w in (Wq, Wk, Wv, Wp))
    common = {
        "WkT8": np.ascontiguousarray(Wk.T * FW).astype(f8),
        "Wv8": (Wv * FW).astype(f8),
        "Wp8": (Wp * FW).astype(f8),
        "Wq16": Wq.astype(b16),
        "Wv16": Wv.astype(b16),
        "Wp16": Wp.astype(b16),
        "bq": np.asarray(bq, f).reshape(1, C),
        "bv": np.asarray(bv, f).reshape(1, C),
        "bp": np.asarray(bp, f).reshape(1, C),
        "gammaT": np.asarray(gamma, f).reshape(C, 1),
        "betaT": np.asarray(beta, f).reshape(C, 1),
        "gsel": _GSEL, "gexp": _GEXP,
        "ones8": np.ones((P, P), f8),
    }
    in_maps = []
    for b in range(B):
        xb = x[b].reshape(HW, C)
        for qb in range(4):
            rolled = np.roll(xb, -qb * QBLK, axis=0)
            m = dict(common)
            m["xT8"] = np.ascontiguousarray(rolled.T).astype(f8)
            m["xn8"] = rolled.astype(f8)
            m["xq"] = np.ascontiguousarray(xb[qb * QBLK:(qb + 1) * QBLK])
            in_maps.append(m)
    return in_maps


def assemble_out(results):
    o = np.empty((B, HW, C), np.float32)
    for b in range(B):
        for qb in range(4):
            o[b, qb * QBLK:(qb + 1) * QBLK] = results[b * 4 + qb]["out"]
    return o.reshape(B, H, W_, C)


_NC_CACHE = {}


def run(inputs, trace=False, trace_cores=None):
    from concourse.bass_utils import run_bass_kernel_spmd
    if "nc" not in _NC_CACHE:
        _NC_CACHE["nc"] = build_kernel()
    nc = _NC_CACHE["nc"]
    in_maps = make_in_maps(**inputs)
    res = run_bass_kernel_spmd(nc, in_maps, core_ids=list(range(8)),
                               trace=trace, trace_cores=trace_cores)
    return assemble_out(res.results), res


def kernel(**inputs) -> np.ndarray:
    out, _ = run(inputs, trace=False)
    return out
